# revision 25
# baseline (speedup 1.0000x reference)
"""Distributed causal multi-head attention on one TRN2 chip (8 NeuronCores).

Problem: B=2, S=2048, D=1024, H=16, DH=64 (f32), causal softmax attention with
QKV + output projections.

Sharding (SPMD, one Bass graph for all 8 cores):
  core i -> batch b = i // 4, head group g = i % 4 (4 of 16 heads).
Each core projects Q/K/V for its 4 heads over the full sequence of its batch
and runs causal attention.  Per-head z (bf16) is AllGathered within each
batch's 4-core group one 512-row band at a time; each core then computes a
256-column slice of the output projection (transposed: out^T[col, q], the
host un-transposes).  Core (b, g) returns out[b, :, 256g:256g+256].

Perf design (343us baseline -> ~238us; bottleneck analysis in comments):
  - host prepacks x/w so SBUF loads are a few wide contiguous DMAs (the
    baseline issued 185 DMAs serially at ~600ns each on the Sync queue,
    gating the first matmul to t=35us; now t=~11us)
  - tiny warmup AllGather bootstraps the CC stream; collectives cannot start
    before the runtime's NEFF-entry barrier (~65us: it absorbs cross-core
    launch skew), so oproj work is scheduled 2+ band-positions after its
    gather fires
  - bands processed in order (0,1,3,2): projections of the next band and
    output projections of gathered bands are emitted as filler quanta inside
    each band's attention steps, keeping the PE stream dense; K/V quanta are
    emitted before Q because band 3's attention consumes band 2's K/V while
    they are being projected
  - exp is batched 2 key-chunks per ACTIVATE ([128,1024] over a 2-bank PSUM
    tile) to amortize the ~350-cycle ACT startup; causal mask is
    multiplicative post-exp; diagonal chunks compute only the causal width
    (start=True zeroes the bank, exp(0)=1 is masked away)
  - softmax normalization: den row -> sbuf copy -> reciprocal_approx_fast
    (the plain DVE reciprocal is 8 cyc/elem and single-lane here: 3.3us) ->
    bf16 -> PE rank-1 ones-matmul broadcast (deferred 2 steps so the PE
    never head-of-line-waits on the DVE chain) -> DVE multiply
    (partition_broadcast on gpsimd contends with the collectives' CC cores;
    a 0-stride-partition DVE operand is rejected by the BIR verifier)
  - the collective stream is serial (~7us fixed cost/op) and later gathers
    absorb inter-core drift (up to ~30us observed), so: one gather per band,
    except the last band which is gathered in halves so the tail exposes
    only the second half; the final band is band 2 (cheapest attention)
  - output projection is transposed (stationary wo half, moving z) to halve
    its matmul count: per-instruction issue overhead (~90ns) dominates
    256-free matmuls
  - PSUM budget (8 banks): psc 2x[128,1024]f32 (4) + pz 2x[128,512] (2) +
    aux 2x[128,512] (2); projection/oproj quanta are self-contained
  - measured residuals: ~90ns/matmul issue overhead (~60us over ~650
    matmuls), inter-core skew (core spans 221-243us), NEFF-entry barrier
"""

import sys

for _p in ("/opt/trn_rl_repo", "/opt/pypackages"):
    if _p not in sys.path:
        sys.path.insert(0, _p)

from collections import deque
from contextlib import ExitStack

import numpy as np

import concourse.bass as bass
import concourse.mybir as mybir
import concourse.tile as tile
from concourse import bacc
from concourse.bass_utils import run_bass_kernel_spmd

B, S, D, H, DH = 2, 2048, 1024, 16, 64
G = 4                       # heads per core
NCORES = 8
SCALE = float(np.sqrt(DH))
TQ = 512                    # query tile (free dim)
NQT = S // TQ               # 4
KC = 128                    # key chunk (partition dim)
DC = 128                    # contraction d-chunk
NDC = D // DC               # 8
EG = G * DH                 # 256: packed head dim per group
VW = DH + 1                 # 65: head slot width in v_aug (ones column)
DS = D // 4                 # 256: output D-column slice per core
XW = NQT * NDC * TQ         # 16384: prepacked x row length
TRIM = True                 # trim diagonal score/AV matmuls to causal width

F32 = mybir.dt.float32
F32R = mybir.dt.float32r
BF16 = mybir.dt.bfloat16

EXP = mybir.ActivationFunctionType.Exp

GROUPS = [[0, 1, 2, 3], [4, 5, 6, 7]]

_CACHE = {}


def _build() -> bass.Bass:
    nc = bacc.Bacc("TRN2", num_devices=NCORES, target_bir_lowering=False)

    xq = nc.declare_dram_parameter("xq", [DC, XW], BF16, isOutput=False)
    xk = nc.declare_dram_parameter("xk", [DC, XW], BF16, isOutput=False)
    xv = nc.declare_dram_parameter("xv", [DC, XW], BF16, isOutput=False)
    wq = nc.declare_dram_parameter("wq", [DC, NDC * EG], BF16, isOutput=False)
    wk = nc.declare_dram_parameter("wk", [DC, NDC * EG], BF16, isOutput=False)
    wv = nc.declare_dram_parameter("wv", [DC, NDC * EG], BF16, isOutput=False)
    wo = nc.declare_dram_parameter("wo", [DC, NDC * DS], BF16, isOutput=False)
    mask = nc.declare_dram_parameter("mask", [KC, G * TQ], BF16, isOutput=False)
    out_ext = nc.declare_dram_parameter("out", [DS, S], F32, isOutput=True)

    with ExitStack() as ctx:
        tc = ctx.enter_context(tile.TileContext(nc))
        const = ctx.enter_context(tc.tile_pool(name="const", bufs=1))
        dram = ctx.enter_context(tc.tile_pool(name="dram", bufs=1, space="DRAM"))
        xpool = ctx.enter_context(tc.tile_pool(name="x", bufs=2))
        epool = ctx.enter_context(tc.tile_pool(name="e", bufs=3))
        rpool = ctx.enter_context(tc.tile_pool(name="r", bufs=2))
        zgpool = ctx.enter_context(tc.tile_pool(name="zg", bufs=2))
        opool = ctx.enter_context(tc.tile_pool(name="o", bufs=2))
        psc_p = ctx.enter_context(tc.tile_pool(name="psc", bufs=2, space="PSUM"))
        pz_p = ctx.enter_context(tc.tile_pool(name="pz", bufs=2, space="PSUM"))
        aux_p = ctx.enter_context(tc.tile_pool(name="aux", bufs=2, space="PSUM"))

        # ---- CC warmup: a tiny AllGather so the collectives stream is
        # bootstrapped while the projections run (first real gather then runs
        # at steady-state speed) ----
        if True:   # CC warmup: absorbs stream bootstrap before first gather
            win = nc.dram_tensor("cc_warm_in", [1, 2], F32)
            wout = nc.dram_tensor("cc_warm_out", [G, 2], F32)
            nc.gpsimd.collective_compute(
                "AllGather",
                mybir.AluOpType.bypass,
                replica_groups=GROUPS,
                ins=[win.ap()],
                outs=[wout.ap()],
            )

        # ---- constants (one wide DMA each; host prepacked) ----
        wq_sb = const.tile([DC, NDC * EG], BF16, name="wq_sb")
        wk_sb = const.tile([DC, NDC * EG], BF16, name="wk_sb")
        wv_sb = const.tile([DC, NDC * EG], BF16, name="wv_sb")
        wo_sb = const.tile([DC, NDC * DS], BF16, name="wo_sb")
        mask_sb = const.tile([KC, G * TQ], BF16, name="mask_sb")

        def load_w(dst, src_, pieces=2):
            wd = dst.shape[1] // pieces
            for i in range(pieces):
                nc.sync.dma_start(dst[:, i * wd:(i + 1) * wd],
                                  src_[:, i * wd:(i + 1) * wd])

        # v_aug: per k-chunk, per head: 64 value cols + 1 ones col
        vaug = const.tile([KC, (S // KC) * G * VW], BF16, name="vaug")
        nc.gpsimd.memset(vaug[:], 1.0)
        ones_b = const.tile([1, DH], BF16, name="ones_b")
        nc.vector.memset(ones_b[:], 1.0)
        zeros_b = const.tile([1, DH], BF16, name="zeros_b")
        nc.vector.memset(zeros_b[:], 0.0)

        q_sb = [const.tile([2 * DH, S], BF16, name=f"q_sb{p}") for p in range(2)]
        k_sb = [const.tile([2 * DH, S], BF16, name=f"k_sb{p}") for p in range(2)]
        z_sb = [const.tile([2 * DH, S], BF16, name=f"z_sb{p}") for p in range(2)]

        # zero the psc banks once: with diagonal trimming, unwritten columns
        # are read by exp (exp(0)=1, then multiplied by the 0 mask)
        for _ in range(2):
            t_ = psc_p.tile([KC, 2 * TQ], F32, tag="psc", name="psc_init")
            nc.vector.memset(t_[:], 0.0)

        # ---- x band staging (double-buffered, one DMA per input per band) ----
        xb = {}

        def load_x_one(nm, src_, t, pieces=4):
            b_ = xpool.tile([DC, NDC * TQ], BF16, tag=f"x{nm}", name=f"x{nm}{t}")
            w4 = NDC * TQ // pieces
            for s4 in range(pieces):
                nc.sync.dma_start(
                    b_[:, s4 * w4:(s4 + 1) * w4],
                    src_[:, t * NDC * TQ + s4 * w4:
                         t * NDC * TQ + (s4 + 1) * w4],
                )
            xb[(nm, t)] = b_

        def load_x_band(t):
            load_x_one("q", xq, t)
            load_x_one("k", xk, t)
            load_x_one("v", xv, t)

        # startup order matches the k-first quantum order
        load_w(wk_sb, wk, pieces=4)
        load_x_one("k", xk, 0)
        load_w(wv_sb, wv)
        load_x_one("v", xv, 0)
        load_w(wq_sb, wq)
        load_x_one("q", xq, 0)
        load_w(wo_sb, wo)
        nc.sync.dma_start(mask_sb[:], mask[:, :])
        load_x_band(1)

        # ---- projection quanta (self-contained: psum alloc + mm + copy) ----
        def q_or_k_quantum(t, p, xkey, wsb, dst):
            def run():
                acc = aux_p.tile([KC, TQ], F32, tag="aux", name="acc")
                xt = xb[(xkey, t)]
                for c in range(NDC):
                    nc.tensor.matmul(
                        acc[:],
                        wsb[:, c * EG + p * 128: c * EG + (p + 1) * 128],
                        xt[:, c * TQ:(c + 1) * TQ],
                        start=(c == 0),
                        stop=(c == NDC - 1),
                    )
                nc.vector.tensor_copy(dst[p][:, t * TQ:(t + 1) * TQ], acc[:])
            return run

        def v_quantum(t, sub):
            def run():
                acc = aux_p.tile([KC, TQ], F32, tag="aux", name="accv")
                xt = xb[("v", t)]
                for c in range(NDC):
                    nc.tensor.matmul(
                        acc[:, 0:EG],
                        xt[:, c * TQ + sub * KC: c * TQ + (sub + 1) * KC],
                        wv_sb[:, c * EG:(c + 1) * EG],
                        start=(c == 0),
                        stop=(c == NDC - 1),
                    )
                kci = t * 4 + sub
                base = kci * G * VW
                dst = vaug[:, base:base + G * VW].rearrange(
                    "p (h w) -> p h w", h=G
                )[:, :, 0:DH]
                src = acc[:, 0:EG].rearrange("p (h w) -> p h w", h=G)
                nc.vector.tensor_copy(dst, src)
            return run

        def proj_quanta(t):
            # K first, then V, then Q: at position 2 the attention of band 3
            # consumes band 2's K (scores, from step 5) and V (AV, step 6+)
            # while these quanta are being popped one per step -- K/V must be
            # emitted before their first consumer or the PE queue deadlocks.
            qs = []
            for p in range(2):
                qs.append(q_or_k_quantum(t, p, "k", wk_sb, k_sb))
            for sub in range(4):
                qs.append(v_quantum(t, sub))
            for p in range(2):
                qs.append(q_or_k_quantum(t, p, "q", wq_sb, q_sb))
            return qs

        # ---- per-band DRAM staging for the z AllGather (one gather per
        # band: the CC stream is serial with ~7us fixed cost per op, so
        # fewer/bigger gathers beat split halves) ----
        zb = [dram.tile([2 * KC, TQ], BF16, name=f"zb{t}") for t in range(NQT)]
        zg = [dram.tile([G * EG, TQ], BF16, name=f"zg{t}") for t in range(NQT)]
        # last-processed band: two half-gathers so the tail only exposes the
        # second (smaller) one
        zbh = [dram.tile([KC, TQ], BF16, name=f"zbh{p}") for p in range(2)]
        zgh = [dram.tile([G * KC, TQ], BF16, name=f"zgh{p}") for p in range(2)]

        def stage_and_gather_half(t, p):
            nc.sync.dma_start(zbh[p][:], z_sb[p][:, t * TQ:(t + 1) * TQ])
            nc.gpsimd.collective_compute(
                "AllGather",
                mybir.AluOpType.bypass,
                replica_groups=GROUPS,
                ins=[zbh[p].opt()],
                outs=[zgh[p].opt()],
            )

        def stage_and_gather(t):
            for p in range(2):
                nc.sync.dma_start(
                    zb[t][p * KC:(p + 1) * KC, :],
                    z_sb[p][:, t * TQ:(t + 1) * TQ],
                )
            nc.gpsimd.collective_compute(
                "AllGather",
                mybir.AluOpType.bypass,
                replica_groups=GROUPS,
                ins=[zb[t].opt()],
                outs=[zg[t].opt()],
            )

        # ---- output projection quanta ----
        def oproj_quanta(t, split=False):
            state = {}

            def first():
                zt = zgpool.tile([KC, NDC * TQ], BF16, tag="zg", name="zg_sb")
                if split:
                    # half p holds e'-chunks {2g(+1)}: chunk c of zg equals
                    # half c%2, group-row c//2
                    for p in range(2):
                        nc.sync.dma_start(
                            zt[:, p * 4 * TQ:(p + 1) * 4 * TQ].rearrange(
                                "p (g j) -> p g j", g=G),
                            zgh[p][:, :].rearrange("(g p) j -> p g j", g=G),
                        )
                else:
                    nc.sync.dma_start(
                        zt[:].rearrange("p (c j) -> p c j", c=NDC),
                        zg[t][:, :].rearrange("(c p) j -> p c j", c=NDC),
                    )
                state["zg"] = zt
                state["o"] = opool.tile([KC, 2 * TQ], F32, tag="o", name="o_sb")

            def half_quantum(ch):
                # transposed oproj: out^T[col, q] = wo_half.T @ z_all; halves
                # the matmul count (2x8 of 512-free vs 4x8 of 256-free) --
                # the 256-free form was ~50% per-instruction issue overhead.
                # out_ext is stored transposed; the host un-transposes.
                def run():
                    if ch == 0:
                        first()
                    zt, o_sb = state["zg"], state["o"]
                    acc = aux_p.tile([KC, TQ], F32, tag="aux", name="acco")
                    for i, c in enumerate((0, 2, 4, 6, 1, 3, 5, 7)):
                        sl = ((c % 2) * 4 + c // 2) if split else c
                        nc.tensor.matmul(
                            acc[:],
                            wo_sb[:, c * DS + ch * KC: c * DS + (ch + 1) * KC],
                            zt[:, sl * TQ:(sl + 1) * TQ],
                            start=(i == 0),
                            stop=(i == NDC - 1),
                        )
                    nc.vector.tensor_copy(
                        o_sb[:, ch * TQ:(ch + 1) * TQ], acc[:]
                    )
                    if ch == 1:
                        nc.sync.dma_start(
                            out_ext[:, t * TQ:(t + 1) * TQ].rearrange(
                                "(c p) j -> p c j", c=2
                            ),
                            o_sb[:].rearrange("p (c j) -> p c j", c=2),
                        )
                return run

            return [half_quantum(ch) for ch in range(2)]

        # ---- attention band with interleaved fillers ----
        def normalize_pre(t, h, pz):
            # DVE-only prefix: den -> 1/den (bf16).  The PE broadcast is
            # deferred ~2 steps so it never head-of-line-stalls the PE queue
            # waiting on this chain (that stall, 16x per run, kept the HAM
            # clock gate cold for the whole kernel).
            den_s = rpool.tile([1, TQ], F32, tag="den", name="den_s")
            nc.vector.tensor_copy(den_s[:], pz[DH:DH + 1, :])
            recip = rpool.tile([1, TQ], F32, tag="recip", name="recip")
            nc.vector.reciprocal_approx_fast(recip[:], den_s[:])
            recip_b = rpool.tile([1, TQ], BF16, tag="recipb", name="recip_b")
            with nc.allow_low_precision(reason="softmax denom recip, bf16"):
                nc.vector.tensor_copy(recip_b[:], recip[:])
            return recip_b

        def normalize_post(t, h, pz, recip_b):
            p_i, off = h // 2, (h % 2) * DH
            pb = aux_p.tile([KC, TQ], F32, tag="aux", name="pb")
            nc.tensor.matmul(
                pb[0:DH, :], ones_b[:], recip_b[:],
                start=True, stop=True,
            )
            bc_s = rpool.tile([DH, TQ], F32, tag="bc", name="bc_s")
            nc.vector.tensor_copy(bc_s[:], pb[0:DH, :])
            nc.vector.tensor_mul(
                z_sb[p_i][off:off + DH, t * TQ:(t + 1) * TQ],
                pz[0:DH, :], bc_s[:]
            )

        def attention_band(t, dq_proj, dq_oproj, split_gather=False):
            nkc = 4 * t + 4
            ngrp = nkc // 2
            steps_total = G * ngrp
            step = 0
            pending = deque()        # (h, g, closure, pz) AV two groups behind
            pending_norm = deque()   # (ready_step, closure)

            def col0(kci):
                dc = kci - 4 * t
                return max(dc, 0) * KC if TRIM else 0

            def pop_pending():
                # AV runs TWO steps behind its scores: exp (~1.15us on ACT)
                # plus the DVE mask multiply must land before the AV reaches
                # the PE queue head, or the PE stalls (measured 83us of PE
                # waits on DVE, 64us on ACT with lag-1)
                ph, pg, pav, ppz = pending.popleft()
                pav()
                if pg == ngrp - 1:    # head ph's last group
                    rb = normalize_pre(t, ph, ppz)

                    def post(ph=ph, ppz=ppz, rb=rb):
                        normalize_post(t, ph, ppz, rb)
                        if split_gather and ph == 1:
                            stage_and_gather_half(t, 0)
                    pending_norm.append((step + 2, post))

            for h in range(G):
                p_i, off = h // 2, (h % 2) * DH
                pz = pz_p.tile([KC, TQ], F32, tag="pz", name=f"pz{h}")
                for g in range(ngrp):
                    # scores for chunks 2g, 2g+1 into a 2-bank psc tile
                    psc = psc_p.tile([KC, 2 * TQ], F32, tag="psc", name="psc")
                    for i in range(2):
                        kci = 2 * g + i
                        c0 = col0(kci)
                        nc.tensor.matmul(
                            psc[:, i * TQ + c0:(i + 1) * TQ],
                            k_sb[p_i][off:off + DH, kci * KC:(kci + 1) * KC],
                            q_sb[p_i][off:off + DH, t * TQ + c0:(t + 1) * TQ],
                            start=True,
                            stop=True,
                        )
                    step += 1
                    e_t = epool.tile([KC, 2 * TQ], BF16, tag="e", name="e_t")
                    nc.scalar.activation(e_t[:], psc[:], EXP)
                    if 2 * g >= 4 * t:      # diagonal pair: multiplicative mask
                        mg = g - 2 * t
                        em = epool.tile(
                            [KC, 2 * TQ], BF16, tag="em", bufs=2, name="em"
                        )
                        # emitted before the filler so it queues ahead of the
                        # filler's psum->sbuf copy on the FIFO DVE
                        nc.vector.tensor_mul(
                            em[:], e_t[:],
                            mask_sb[:, mg * 2 * TQ:(mg + 1) * 2 * TQ],
                        )
                        e_use = em
                    else:
                        e_use = e_t
                    if dq_proj:
                        dq_proj.popleft()()
                    elif dq_oproj and step > (3 * steps_total) // 4:
                        dq_oproj.popleft()()

                    def av(h=h, g=g, e_use=e_use, pz=pz):
                        for i in range(2):
                            kci = 2 * g + i
                            c0 = col0(kci)
                            nc.tensor.matmul(
                                pz[0:VW, c0:TQ],
                                vaug[:, kci * G * VW + h * VW:
                                     kci * G * VW + (h + 1) * VW],
                                e_use[:, i * TQ + c0:(i + 1) * TQ],
                                start=(kci == 0),
                                stop=(kci == nkc - 1),
                            )

                    while pending_norm and step >= pending_norm[0][0]:
                        pending_norm.popleft()[1]()
                    if len(pending) == 2:
                        pop_pending()
                    pending.append((h, g, av, pz))

            # flush both lagged AVs + normalizes, staging, leftovers
            while pending:
                pop_pending()
            for dq in (dq_proj, dq_oproj):
                if dq and pending_norm:
                    dq.popleft()()
            while pending_norm:
                pending_norm.popleft()[1]()
            if split_gather:
                stage_and_gather_half(t, 1)
            else:
                stage_and_gather(t)
            for dq in (dq_proj, dq_oproj):
                while dq:
                    dq.popleft()()

        # ---- band order: cheap band 2 last, so the final gather triggers
        # as early as possible and band 2's attention hosts late oproj work.
        # pos0 proj'd in prologue; during pos i we project band at pos i+1;
        # oproj(band) runs two positions after its gathers fire. ----
        BAND_ORDER = (0, 1, 3, 2)
        for qm in proj_quanta(BAND_ORDER[0]):
            qm()

        for i, t in enumerate(BAND_ORDER):
            if 1 <= i < NQT - 1:
                load_x_band(BAND_ORDER[i + 1])
            dq_proj = (deque(proj_quanta(BAND_ORDER[i + 1]))
                       if i + 1 < NQT else deque())
            dq_oproj = deque()
            if i == 2:
                dq_oproj.extend(oproj_quanta(BAND_ORDER[0]))
            elif i == 3:
                dq_oproj.extend(oproj_quanta(BAND_ORDER[1]))
            attention_band(t, dq_proj, dq_oproj,
                           split_gather=(i == NQT - 1))

        # ---- tail: band 3's oproj runs while band 2's gather is in flight,
        # then band 2's oproj ----
        for qm in oproj_quanta(BAND_ORDER[2]):
            qm()
        for qm in oproj_quanta(BAND_ORDER[3], split=True):
            qm()

    nc.compile()
    return nc


def _get_graph() -> bass.Bass:
    if "nc" not in _CACHE:
        _CACHE["nc"] = _build()
    return _CACHE["nc"]


def _make_mask() -> np.ndarray:
    # multiplicative causal mask for the 4 diagonal chunks of a band:
    # m[x, dc*TQ + y] = 1.0 where key dc*KC+x <= query y else 0.0
    m = np.zeros((KC, G * TQ), np.float32)
    x = np.arange(KC)[:, None]
    y = np.arange(TQ)[None, :]
    for dc in range(G):
        m[:, dc * TQ:(dc + 1) * TQ] = (dc * KC + x <= y).astype(np.float32)
    return m


def _make_in_maps(inputs: dict) -> list[dict]:
    import ml_dtypes

    bf16 = ml_dtypes.bfloat16
    qx = np.asarray(inputs["query_input"], np.float32)
    kx = np.asarray(inputs["key_input"], np.float32)
    vx = np.asarray(inputs["value_input"], np.float32)
    WQ = (np.asarray(inputs["W_Q"], np.float32) / SCALE).astype(bf16)
    WK = np.asarray(inputs["W_K"], np.float32).astype(bf16)
    WV = np.asarray(inputs["W_V"], np.float32).astype(bf16)
    WO = np.asarray(inputs["W_O"], np.float32).astype(bf16)

    mask = _make_mask().astype(bf16)

    # x prepack: x_prep[p, (t*NDC + c)*TQ + j] = x[b, t*TQ + j, c*DC + p]
    def prep_x(arr, b):
        a = arr[b].astype(bf16)                       # [S, D]
        a = a.reshape(NQT, TQ, NDC, DC)               # [t, j, c, p]
        a = a.transpose(3, 0, 2, 1)                   # [p, t, c, j]
        return np.ascontiguousarray(a.reshape(DC, XW))

    xT = {
        (nm, b): prep_x(arr, b)
        for nm, arr in (("xq", qx), ("xk", kx), ("xv", vx))
        for b in range(B)
    }

    # weights: w_prep[p, c*EG + m] = W2[c*DC + p, m], W2 = [D, EG] head-packed
    def prep_w(w, hs):
        W2 = w[hs].transpose(1, 0, 2).reshape(D, EG)  # [d, h*DH + e]
        W2 = W2.reshape(NDC, DC, EG).transpose(1, 0, 2)
        return np.ascontiguousarray(W2.reshape(DC, NDC * EG))

    WO_flat = WO.reshape(H * DH, D)   # e' = h*64 + e, h-major (AllGather order)
    wmaps = []
    for g in range(G):
        hs = slice(g * G, (g + 1) * G)
        wo_slice = WO_flat[:, g * DS:(g + 1) * DS]    # [D, DS]
        wo_prep = np.ascontiguousarray(
            wo_slice.reshape(NDC, DC, DS).transpose(1, 0, 2).reshape(
                DC, NDC * DS)
        )
        wmaps.append(
            {
                "wq": prep_w(WQ, hs),
                "wk": prep_w(WK, hs),
                "wv": prep_w(WV, hs),
                "wo": wo_prep,
            }
        )

    in_maps = []
    for core in range(NCORES):
        b, g = core // G, core % G
        m = {
            "xq": xT[("xq", b)],
            "xk": xT[("xk", b)],
            "xv": xT[("xv", b)],
            "mask": mask,
        }
        m.update(wmaps[g])
        in_maps.append(m)
    return in_maps


def _assemble(results: list[dict]) -> np.ndarray:
    out = np.empty((B, S, D), np.float32)
    for core in range(NCORES):
        b, g = core // G, core % G
        out[b, :, g * DS:(g + 1) * DS] = results[core]["out"].T
    return out


def run(inputs: dict, trace: bool = False):
    """Run on hardware; returns (output, BassKernelResults)."""
    nc = _get_graph()
    res = run_bass_kernel_spmd(
        nc, _make_in_maps(inputs), core_ids=list(range(NCORES)), trace=trace
    )
    return _assemble(res.results), res


def kernel(**inputs) -> np.ndarray:
    out, _ = run(inputs)
    return out
